# revision 25
# baseline (speedup 1.0000x reference)
"""CPCNet forward on 8 Trainium2 NeuronCores (Bass/Tile).

Data-parallel over batch: each of the 8 cores processes 16 of the 128
batch elements end-to-end (embed GEMM -> GRU over 16 context windows ->
bilinear scoring), parameters replicated. No collectives needed.

Per-core layout (all "transposed" space, embed dim on partitions):
  rows = flattened [C*T]-windows: Xc 256 (s*16+b), Xp 256 (s*16+b),
  Xb 2560 (nb*256 + s*16 + b).  ET[sbuf] = [100, 3072] embeddings^T.

Embed GEMM (the memory-bound bulk, ~103 MB/core, HW-measured ~370 us
wall for the whole net): X streams in natural layout [128 rows, k]
(fully contiguous DMA) and is cast f32->bf16 inside the SWDGE load DMAs;
PE transposes 128x128 bf16 blocks into PSUM (1 cyc/row vs 2-pass fp32);
DVE+ACT evacuate pairs of transposed chunks to SBUF; the PE accumulates
W_chunk.T @ X^T into E^T[100, 512] per 512-row block in bf16.

GRU + bilinear are fp32 and overlap the Xb embed stream (steps spread
between slabs; elementwise on the otherwise-idle GpSimd so the embed's
PSUM-evacuation copies never queue behind the GRU's serial chain).
Bilinear uses broadcast-multiply + ones-matmul column reduction to avoid
per-batch diagonal extraction; the final reduction runs as float32r.
"""

import numpy as np

import concourse.bacc as bacc
import concourse.mybir as mybir
import concourse.tile as tile
from concourse.bass_utils import run_bass_kernel_spmd

N_CORES = 8
BC = 16          # batch per core
NE = 16          # context windows (gru seq len)
NB = 10          # negative samples
CT = 8400        # flattened window (21*400)
E = 100          # embed dim == gru hidden
ROWS = BC * NE * (2 + NB)   # 3072 rows per core
NBLK = ROWS // 512          # 6 blocks of 512 rows
SLABS = [(8192, 208), (0, 2048), (2048, 2048), (4096, 2048), (6144, 2048)]
NCHUNK = 66                 # ceil(8400/128); last chunk is 80 wide

F32 = mybir.dt.float32
BF16 = mybir.dt.bfloat16

# The embed X pipeline runs in bf16: the f32->bf16 cast happens inside the
# SWDGE (gpsimd) load DMAs, so transposes and the embed matmul stream at
# 1 cyc/row on the PE (fp32 transposes measured 325 ns each = 515 us/core;
# bf16 ~3x cheaper).  HW-measured rel err of the bf16 embed ~2.4e-3.
# GRU + bilinear stay fp32.


def _block_src(Xc, Xp, Xb, blk, st, k0, kw):
    """DRAM source AP for 128-row subtile `st` of 512-row block `blk`,
    k-range [k0, k0+kw). Row order within subtile: (s, b), s-major."""
    if blk == 0:
        base = Xc if st < 2 else Xp
        sh = st % 2
        return base[:, sh * 8:(sh + 1) * 8, k0:k0 + kw].transpose([1, 0, 2])
    nb = 2 * (blk - 1) + st // 2
    sh = st % 2
    return Xb[:, sh * 8:(sh + 1) * 8, nb, k0:k0 + kw].transpose([1, 0, 2])


def _emit(nc, tc, ctx):
    Xc = nc.dram_tensor("Xc", [BC, NE, CT], F32, kind="ExternalInput").ap()
    Xp = nc.dram_tensor("Xp", [BC, NE, CT], F32, kind="ExternalInput").ap()
    Xb = nc.dram_tensor("Xb", [BC, NE, NB, CT], F32, kind="ExternalInput").ap()
    Wemb = nc.dram_tensor("Wemb", [128, NCHUNK * E], BF16,
                          kind="ExternalInput").ap()
    bemb = nc.dram_tensor("bemb", [E, 1], F32, kind="ExternalInput").ap()
    WihT = nc.dram_tensor("WihT", [E, 300], F32, kind="ExternalInput").ap()
    WhhT = nc.dram_tensor("WhhT", [E, 300], F32, kind="ExternalInput").ap()
    bias4 = nc.dram_tensor("bias4", [E, 4], F32, kind="ExternalInput").ap()
    Wbil = nc.dram_tensor("Wbil", [E, NE * E], F32, kind="ExternalInput").ap()
    ident = nc.dram_tensor("ident", [128, 128], BF16, kind="ExternalInput").ap()
    ones = nc.dram_tensor("ones", [E, 1], mybir.dt.float32r,
                          kind="ExternalInput").ap()
    out_d = nc.dram_tensor("out", [1, NE * BC * (NB + 1)], F32,
                           kind="ExternalOutput").ap()

    P = ctx.enter_context  # pools

    const = P(tc.tile_pool(name="const", bufs=1))
    xnat = P(tc.tile_pool(name="xnat", bufs=4))
    xtp = P(tc.tile_pool(name="xt", bufs=4))
    psT = P(tc.tile_pool(name="psT", bufs=3, space="PSUM"))
    psE = P(tc.tile_pool(name="psE", bufs=1, space="PSUM"))
    psS = P(tc.tile_pool(name="psS", bufs=1, space="PSUM"))
    small = P(tc.tile_pool(name="small", bufs=2))

    # ---- persistent SBUF ----
    # identity first: the very first transposes only need id_sb + one X slab
    id_sb = const.tile([128, 128], BF16)
    nc.sync.dma_start(id_sb[:], ident[:])
    # W_embed arrives pre-chunked [128, 66*100] and pre-cast to bf16 from
    # the host: one fully-contiguous 1.7 MB DMA, no on-chip cast, so the
    # first embed matmul is ready ~immediately.
    W_sb = const.tile([128, NCHUNK * E], BF16)
    nc.sync.dma_start(W_sb[:], Wemb[:])
    bemb_sb = const.tile([E, 1], F32)
    nc.scalar.dma_start(bemb_sb[:], bemb[:])
    WihT_sb = const.tile([E, 300], F32)
    nc.scalar.dma_start(WihT_sb[:], WihT[:])
    WhhT_sb = const.tile([E, 300], F32)
    nc.scalar.dma_start(WhhT_sb[:], WhhT[:])
    bias4_sb = const.tile([E, 4], F32)
    nc.scalar.dma_start(bias4_sb[:], bias4[:])
    Wbil_sb = const.tile([E, NE * E], F32)
    nc.scalar.dma_start(Wbil_sb[:], Wbil[:])
    ones_sb = const.tile([E, 1], mybir.dt.float32r)
    nc.scalar.dma_start(ones_sb[:], ones[:])

    ET = const.tile([E, ROWS], F32)                # all embeddings, transposed
    gi_sb = const.tile([E, NE * 3 * BC], F32)      # preacts, [s][r|z|n] blocks
    h = const.tile([E, BC], F32)                   # GRU hidden state (h^T)
    tmp_all = const.tile([E, NE * BC * (NB + 1)], mybir.dt.float32r)
    out_sb = const.tile([1, NE * BC * (NB + 1)], F32)

    gi_v = gi_sb.rearrange("e (s g b) -> e s g b", s=NE, g=3)

    def gru_init():
        # gi preacts for all 16 steps in 3 gate matmuls; biases folded
        # (r,z get b_ih+b_hh; n gets b_ih only).  Scattered into the
        # per-step-interleaved gi layout so each step reads one slice.
        nc.vector.memset(h[:], 0.0)
        for g in range(3):
            gp = psS.tile([E, NE * BC], F32, tag="sp0", name="gp")
            nc.tensor.matmul(gp[:, :], WihT_sb[:, g * E:(g + 1) * E],
                             ET[:, 0:NE * BC], start=True, stop=True)
            nc.scalar.add(gi_v[:, :, g, :],
                          gp.rearrange("e (s b) -> e s b", s=NE),
                          bias4_sb[:, g:g + 1])

    def gru_step(s):
        # DVE only evacuates gh (1 op); elementwise on the idle GpSimd,
        # sigmoid/tanh on ACT -- keeps the embed pair-copies from
        # head-of-line blocking behind the GRU's serial chain.
        c0 = s * 3 * BC
        gh = psS.tile([E, 3 * BC], F32, tag="sp1", name="gh")
        for g in range(3):
            nc.tensor.matmul(gh[:, g * BC:(g + 1) * BC],
                             WhhT_sb[:, g * E:(g + 1) * E], h[:],
                             start=True, stop=True)
        ghs = small.tile([E, 3 * BC], F32, tag="ghs", name="ghs")
        nc.vector.tensor_copy(ghs[:], gh[:])
        rzt = small.tile([E, 2 * BC], F32, tag="rzt", name="rzt")
        nc.gpsimd.tensor_add(rzt[:], ghs[:, 0:2 * BC], gi_sb[:, c0:c0 + 2 * BC])
        rz = small.tile([E, 2 * BC], F32, tag="rz", name="rz")
        nc.scalar.activation(rz[:], rzt[:],
                             mybir.ActivationFunctionType.Sigmoid)
        hn = small.tile([E, BC], F32, tag="hn", name="hn")
        nc.gpsimd.tensor_scalar_add(hn[:], ghs[:, 2 * BC:3 * BC],
                                    bias4_sb[:, 3:4])  # gh_n + b_hn
        t1 = small.tile([E, BC], F32, tag="t1", name="t1")
        nc.gpsimd.tensor_mul(t1[:], rz[:, 0:BC], hn[:])
        t2 = small.tile([E, BC], F32, tag="t2", name="t2")
        nc.gpsimd.tensor_add(t2[:], t1[:], gi_sb[:, c0 + 2 * BC:c0 + 3 * BC])
        n = small.tile([E, BC], F32, tag="n", name="n")
        nc.scalar.activation(n[:], t2[:], mybir.ActivationFunctionType.Tanh)
        d = small.tile([E, BC], F32, tag="d", name="d")
        nc.gpsimd.tensor_sub(d[:], h[:], n[:])
        zd = small.tile([E, BC], F32, tag="zd", name="zd")
        nc.gpsimd.tensor_mul(zd[:], rz[:, BC:2 * BC], d[:])
        nc.gpsimd.tensor_add(h[:], n[:], zd[:])    # h = n + z*(h-n)

    # ---- embed: 6 blocks of 512 rows; GRU interleaved after block 0 ----
    for blk in range(NBLK):
        et = psE.tile([E, 512], F32)
        nmm = 0
        for si, (k0, kw) in enumerate(SLABS):
            # one GRU step between slabs (blocks 2..5 handle steps 0..15;
            # block 1 runs gru_init emitted at the block-0 boundary)
            if 2 <= blk <= 5 and si < 4:
                gru_step(4 * (blk - 2) + si)
            xs = [xnat.tile([128, 2048], BF16, tag=f"xn{st}", name=f"xn{st}")
                  for st in range(4)]
            for st in range(4):
                # gpsimd SWDGE casts f32 -> bf16 in the DMA
                nc.gpsimd.dma_start(xs[st][:, 0:kw],
                                    _block_src(Xc, Xp, Xb, blk, st, k0, kw))
            nj = (kw + 127) // 128
            assert nj % 2 == 0
            jbase = k0 // 128
            for jp in range(nj // 2):
                pt = psT.tile([128, 1024], BF16)
                kjs = []
                for u in range(2):
                    j = jp * 2 + u
                    kj = min(128, kw - j * 128)
                    kjs.append(kj)
                    for st in range(4):
                        nc.tensor.transpose(
                            pt[0:kj, u * 512 + st * 128:u * 512 + (st + 1) * 128],
                            xs[st][:, j * 128:j * 128 + kj],
                            id_sb[:])
                xt = xtp.tile([128, 1024], BF16)
                if kjs[1] == 128:
                    nc.vector.tensor_copy(xt[:, 0:640], pt[:, 0:640])
                    nc.scalar.copy(xt[:, 640:1024], pt[:, 640:1024])
                else:  # last pair: u=1 chunk only has kjs[1] valid rows
                    nc.vector.tensor_copy(xt[:, 0:512], pt[:, 0:512])
                    nc.scalar.copy(xt[0:kjs[1], 512:1024], pt[0:kjs[1], 512:1024])
                for u in range(2):
                    jg = jbase + jp * 2 + u
                    nc.tensor.matmul(
                        et[:, :],
                        W_sb[0:kjs[u], jg * E:(jg + 1) * E],
                        xt[0:kjs[u], u * 512:u * 512 + 512],
                        start=(nmm == 0), stop=(nmm == NCHUNK - 1),
                        skip_group_check=True)
                    nmm += 1
        # bias + evacuate to ET
        nc.scalar.add(ET[:, blk * 512:(blk + 1) * 512], et[:, :],
                      bemb_sb[:, 0:1])
        # gi preacts as soon as block 0 (Ec) is done
        if blk == 0:
            gru_init()

    # ---- bilinear scores ----
    tmp_v = tmp_all.rearrange("e (s b p) -> e s b p", s=NE, b=BC)
    Eb_v = ET[:, 512:ROWS].rearrange("e (nb s b) -> e nb s b", nb=NB, s=NE)
    for s in range(NE):
        Ap = psS.tile([E, BC], F32, tag="bilA", name="Ap", bufs=2)
        nc.tensor.matmul(Ap[:, :], Wbil_sb[:, s * E:(s + 1) * E], h[:],
                         start=True, stop=True)  # A_s^T = W_bil[s].T @ h^T
        nc.vector.tensor_mul(tmp_v[:, s, :, 0],
                             ET[:, NE * BC + s * BC: NE * BC + (s + 1) * BC],
                             Ap[:])
        nc.vector.tensor_mul(
            tmp_v[:, s, :, 1:NB + 1].rearrange("e b p -> e p b"),
            Eb_v[:, :, s, :],
            Ap[:].unsqueeze(1).broadcast_to([E, NB, BC]))
    TOT = NE * BC * (NB + 1)
    for c0 in range(0, TOT, 512):
        w = min(512, TOT - c0)
        rp = psS.tile([1, 512], F32, tag="sp1")
        nc.tensor.matmul(rp[0:1, 0:w], ones_sb[:, 0:1], tmp_all[:, c0:c0 + w],
                         start=True, stop=True)
        nc.scalar.copy(out_sb[:, c0:c0 + w], rp[0:1, 0:w])
    nc.sync.dma_start(out_d[:], out_sb[:])


def build():
    import contextlib
    nc = bacc.Bacc("TRN2", target_bir_lowering=False, debug=False,
                   enable_asserts=False, num_devices=N_CORES)
    with tile.TileContext(nc) as tc:
        with contextlib.ExitStack() as ctx:
            _emit(nc, tc, ctx)
    nc.compile()
    return nc


_NC = None


def make_in_maps(Xc, Xp, Xb, W_embed, b_embed, W_ih, W_hh, b_ih, b_hh, W_bil):
    B = Xc.shape[0]
    Xc_r = np.ascontiguousarray(Xc, np.float32).reshape(B, NE, CT)
    Xp_r = np.ascontiguousarray(Xp, np.float32).reshape(B, NE, CT)
    Xb_r = np.ascontiguousarray(Xb, np.float32).reshape(B, NE, NB, CT)

    import ml_dtypes
    W_embed = np.ascontiguousarray(W_embed, np.float32)
    W_ch = np.zeros((128, NCHUNK * E), np.float32)
    for j in range(NCHUNK):
        kj = min(128, CT - j * 128)
        W_ch[:kj, j * E:(j + 1) * E] = W_embed[j * 128:j * 128 + kj]
    W_ch = W_ch.astype(ml_dtypes.bfloat16)
    bemb = np.ascontiguousarray(b_embed, np.float32).reshape(E, 1)
    WihT = np.ascontiguousarray(W_ih.T, np.float32)          # [100, 300]
    WhhT = np.ascontiguousarray(W_hh.T, np.float32)
    bias4 = np.stack([b_ih[0:E] + b_hh[0:E],
                      b_ih[E:2 * E] + b_hh[E:2 * E],
                      b_ih[2 * E:3 * E],
                      b_hh[2 * E:3 * E]], axis=1).astype(np.float32)
    Wbil_r = np.ascontiguousarray(
        np.transpose(W_bil, (1, 0, 2)).reshape(E, NE * E), np.float32)
    ident = np.eye(128).astype(ml_dtypes.bfloat16)
    ones = np.ones((E, 1), np.float32)

    shared = dict(Wemb=W_ch, bemb=bemb, WihT=WihT, WhhT=WhhT,
                  bias4=bias4, Wbil=Wbil_r, ident=ident, ones=ones)
    in_maps = []
    for c in range(N_CORES):
        sl = slice(c * BC, (c + 1) * BC)
        in_maps.append(dict(Xc=Xc_r[sl], Xp=Xp_r[sl], Xb=Xb_r[sl], **shared))
    return in_maps


def gather(results):
    outs = []
    for c in range(N_CORES):
        o = results[c]["out"].reshape(NE, BC, NB + 1)       # [s, b, p]
        outs.append(np.transpose(o, (1, 0, 2)))             # [b, s, p]
    return np.concatenate(outs, axis=0).astype(np.float32)  # [128, 16, 11]


def kernel(Xc, Xp, Xb, W_embed, b_embed, W_ih, W_hh, b_ih, b_hh, W_bil):
    global _NC
    if _NC is None:
        _NC = build()
    in_maps = make_in_maps(Xc, Xp, Xb, W_embed, b_embed, W_ih, W_hh,
                           b_ih, b_hh, W_bil)
    res = run_bass_kernel_spmd(_NC, in_maps, core_ids=list(range(N_CORES)))
    return gather(res.results)
